# revision 1
# baseline (speedup 1.0000x reference)
"""Batch-assign-probability (VQ codebook softmax) kernel for 8 Trainium2 cores.

Math: for each valid row x (D=512), over K=256 centers c_k:
    softmax_k(-||x - c_k||^2) == softmax_k(2 x.c_k - ||c_k||^2)
(the ||x||^2 term is constant over k and cancels in softmax).

Sharding: batch B=16 split across 8 cores (2 batches = 2048 valid rows per
core); the small centers table is replicated. Host prep: slice the valid
(unmasked) timesteps, transpose x to [D, rows] so the contraction dim lands
on SBUF partitions, fold the 2x scale into ct = (2*centers)^T, and split
x / ct into bf16 hi+lo pairs for a 3-pass full-rate matmul:
    x.ct ~= xh.cth + xh.ctl + xl.cth        (error ~2e-4 relative)
The -||c||^2 bias is folded in as one contraction-dim-3 matmul against a
3-level bf16 split of the bias (ones rows on the x side). Host packs each
DMA's source region fully contiguous (8KB-per-partition runs).

Device (per core, Tile framework):
  - load ct hi/lo + bias once; stream x hi/lo in row-groups (small first
    group so the PE starts early, small last group so the exposed softmax
    tail is short); per 128-row tile: 12 bf16 matmuls + 1 bias matmul ->
    PSUM logits [128,256]; reduce_max(negate) -> ACT exp(bias=-max,
    accum sum) -> reciprocal -> scale -> group out DMA.
"""

import numpy as np
import ml_dtypes

import concourse.bacc as bacc
import concourse.tile as tile
from concourse import mybir
from concourse.bass_utils import run_bass_kernel_spmd

B, T, W, C, K = 16, 2048, 512, 1, 256
VALID_T = 1024
D = W * C                       # 512
N_CORES = 8
B_PER_CORE = B // N_CORES       # 2
ROWS = B_PER_CORE * VALID_T     # 2048 rows per core
P = 128
D_CHUNKS = D // P               # 4
GROUPS = [128, 256, 512, 512, 384, 128, 128]   # rows per x/out DMA group
N_WARM_MM = 8                  # dummy matmuls to lift the PE HAM clock-gate
assert sum(GROUPS) == ROWS
X_TOTAL = P * 2 * D_CHUNKS * ROWS    # flat bf16 element count of x param

BF16_NP = ml_dtypes.bfloat16

_CACHE: dict = {}


def _build_bass():
    f32 = mybir.dt.float32
    bf16 = mybir.dt.bfloat16
    nc = bacc.Bacc()
    # x hi/lo, group-major, fully contiguous per group: for each group g
    # (R rows), block [128p, 2h, 4c, R] flattened.
    xp = nc.declare_dram_parameter("xp", [X_TOTAL], bf16, isOutput=False)
    # ct hi block then lo block, each [128p, 4c, 256k] contiguous.
    ctp = nc.declare_dram_parameter("ctp", [2 * P * D_CHUNKS * K], bf16,
                                    isOutput=False)
    bias3 = nc.declare_dram_parameter("bias3", [P, K], bf16, isOutput=False)
    ones3 = nc.declare_dram_parameter("ones3", [P, P], bf16, isOutput=False)
    out = nc.declare_dram_parameter("out", [ROWS, K], f32, isOutput=True)

    out_v = out.rearrange("(t p) k -> p t k", p=P)       # [128, 16, 256]
    ct_half = P * D_CHUNKS * K

    with tile.TileContext(nc) as tc:
        with (
            tc.tile_pool(name="singles", bufs=1) as singles,
            tc.tile_pool(name="xpool", bufs=1) as xpool,
            tc.tile_pool(name="opool", bufs=3) as opool,
            tc.tile_pool(name="small", bufs=8) as small,
            tc.tile_pool(name="psum", bufs=7, space="PSUM") as psum,
            tc.tile_pool(name="psum_warm", bufs=1, space="PSUM") as psum_warm,
        ):
            # Two HWDGE rings in parallel: the scalar ring carries the small
            # constant loads (ct hi/lo, bias, ones) while the sync ring
            # carries only the x groups — the first x wire overlaps ct's
            # instead of queueing behind it in the FIFO.
            ct_sb = singles.tile([P, 2, D_CHUNKS, K], bf16)
            xgs = []
            xoff = 0

            def x_dma(g, R, split=False, eng=None):
                xg = xpool.tile([P, 2, D_CHUNKS, R], bf16, tag=f"xg{g}")
                n = P * 2 * D_CHUNKS * R
                src = xp[xoff:xoff + n].rearrange(
                    "(p h c r) -> p h c r", p=P, h=2, c=D_CHUNKS)
                if eng is not None:
                    eng.dma_start(out=xg[:], in_=src)
                elif split:
                    # hi half on the sync ring (gates the first matmuls),
                    # lo half in parallel on the scalar ring
                    nc.sync.dma_start(out=xg[:, 0], in_=src[:, 0])
                    nc.scalar.dma_start(out=xg[:, 1], in_=src[:, 1])
                else:
                    nc.sync.dma_start(out=xg[:], in_=src)
                xgs.append(xg)
                return n

            def ct_dma(h, eng=None):
                (eng or nc.sync).dma_start(
                    out=ct_sb[:, h],
                    in_=ctp[h * ct_half:(h + 1) * ct_half].rearrange(
                        "(p c k) -> p c k", p=P, c=D_CHUNKS),
                )

            ct_dma(0)
            xoff += x_dma(0, GROUPS[0], split=True)
            ct_dma(1)
            bias_sb = singles.tile([P, K], bf16)
            nc.scalar.dma_start(out=bias_sb[:], in_=bias3[:])
            ones_sb = singles.tile([P, P], bf16)
            nc.scalar.dma_start(out=ones_sb[:], in_=ones3[:])
            for g, R in enumerate(GROUPS[1:], start=1):
                xoff += x_dma(g, R, split=(g == 1))

            # PE warm-up: dummy matmuls on scratch data keep the PE busy
            # through the HAM activity window while the first x DMA lands,
            # so the real matmul stream runs at 2.4 GHz from the start.
            warm_sb = singles.tile([P, 512], bf16)
            nc.gpsimd.memset(warm_sb[:], 0.0)
            warm_ps = psum_warm.tile([P, 512], f32, tag="warm")
            for _ in range(N_WARM_MM):
                nc.tensor.matmul(
                    warm_ps[:], lhsT=warm_sb[:, :P], rhs=warm_sb[:],
                    start=True, stop=True,
                )

            t0 = 0  # running 128-row tile index
            for g, R in enumerate(GROUPS):
                xg = xgs[g]
                subtiles = R // P
                og = opool.tile([P, subtiles, K], f32, tag="og")
                esum_g = small.tile([P, subtiles], f32, tag="esum")
                # pair subtiles into one full PSUM bank: shared reduce_max,
                # per-group reciprocal + broadcast multiply
                for s0 in range(0, subtiles, 2):
                    pair = min(2, subtiles - s0)
                    ps = psum.tile([P, pair, K], f32, tag="ps")
                    for j in range(pair):
                        s = s0 + j
                        rsl = slice(s * P, (s + 1) * P)
                        first = True
                        for xh_i, ct_i in ((0, 0), (1, 0), (0, 1)):
                            for c in range(D_CHUNKS):
                                nc.tensor.matmul(
                                    ps[:, j, :],
                                    lhsT=xg[:, xh_i, c, rsl],
                                    rhs=ct_sb[:, ct_i, c, :],
                                    start=first,
                                    stop=False,
                                )
                                first = False
                        nc.tensor.matmul(
                            ps[:, j, :],
                            lhsT=ones_sb[:, :],
                            rhs=bias_sb[:, :],
                            start=False,
                            stop=True,
                        )
                    negm = small.tile([P, pair], f32, tag="negm")
                    nc.vector.reduce_max(
                        out=negm[:], in_=ps[:], axis=mybir.AxisListType.X, negate=True
                    )
                    for j in range(pair):
                        nc.scalar.activation(
                            out=og[:, s0 + j, :],
                            in_=ps[:, j, :],
                            func=mybir.ActivationFunctionType.Exp,
                            bias=negm[:, j:j + 1],
                            scale=1.0,
                            accum_out=esum_g[:, s0 + j:s0 + j + 1],
                        )
                rinv_g = small.tile([P, subtiles], f32, tag="rinv")
                nc.vector.reciprocal(out=rinv_g[:], in_=esum_g[:])
                nc.vector.tensor_mul(
                    og[:],
                    og[:],
                    rinv_g[:, :, None].broadcast_to([P, subtiles, K]),
                )
                nc.sync.dma_start(out=out_v[:, t0:t0 + subtiles, :], in_=og[:])
                t0 += subtiles
    nc.finalize()
    return nc


def get_nc():
    if "nc" not in _CACHE:
        _CACHE["nc"] = _build_bass()
    return _CACHE["nc"]


def _split_hi_lo(a: np.ndarray) -> tuple[np.ndarray, np.ndarray]:
    hi = a.astype(BF16_NP)
    lo = (a - hi.astype(np.float32)).astype(BF16_NP)
    return hi, lo


def prep_inputs(y_pred: np.ndarray, mask: np.ndarray, centers: np.ndarray):
    """Host-side shard prep: valid-timestep slice, per-core transpose,
    bf16 hi/lo splits, contiguous per-DMA packing."""
    x = np.ascontiguousarray(y_pred.reshape(B, T, D))
    masktime = np.asarray(mask).reshape(B, T, D)[0, :, 0]
    valid_idx = np.nonzero(masktime == 0)[0][:VALID_T]
    assert valid_idx.shape[0] == VALID_T
    if valid_idx[0] == 0 and valid_idx[-1] == VALID_T - 1:
        xv = x[:, :VALID_T]                    # [B, VALID_T, D]
    else:
        xv = x[:, valid_idx]

    centers = np.asarray(centers, dtype=np.float32)
    cth, ctl = _split_hi_lo((2.0 * centers).T)              # [D, K] each
    # [h, c, p, k] -> [h, p, c, k] contiguous
    ct_blocks = [
        np.ascontiguousarray(h.reshape(D_CHUNKS, P, K).transpose(1, 0, 2)).ravel()
        for h in (cth, ctl)
    ]
    ctp = np.ascontiguousarray(np.concatenate(ct_blocks))

    negc2 = -(centers.astype(np.float64) ** 2).sum(axis=1)  # [K]
    b1 = negc2.astype(BF16_NP)
    r1 = negc2 - b1.astype(np.float64)
    b2 = r1.astype(BF16_NP)
    b3 = (r1 - b2.astype(np.float64)).astype(BF16_NP)
    # contraction padded to 128 rows so the bias matmul's weight load
    # matches the regular [128,128] shape (keeps PE weight pipelining)
    bias3 = np.zeros((P, K), dtype=BF16_NP)
    bias3[0], bias3[1], bias3[2] = b1, b2, b3
    ones3 = np.ones((P, P), dtype=BF16_NP)

    in_maps = []
    for core in range(N_CORES):
        xc = xv[core * B_PER_CORE:(core + 1) * B_PER_CORE].reshape(ROWS, D)
        xTc = np.ascontiguousarray(xc.T)                    # [D, ROWS]
        xh, xl = _split_hi_lo(xTc)
        # [h, c, p, row] -> [p, h, c, row]
        base = np.stack([xh, xl]).reshape(2, D_CHUNKS, P, ROWS).transpose(2, 0, 1, 3)
        blocks = []
        r0 = 0
        for R in GROUPS:
            blocks.append(np.ascontiguousarray(base[:, :, :, r0:r0 + R]).ravel())
            r0 += R
        xp = np.concatenate(blocks)
        assert xp.shape[0] == X_TOTAL
        in_maps.append({"xp": xp, "ctp": ctp, "bias3": bias3, "ones3": ones3})
    return in_maps


def kernel(y_pred: np.ndarray, mask: np.ndarray, centers: np.ndarray,
           **run_kwargs) -> np.ndarray:
    in_maps = prep_inputs(y_pred, mask, centers)
    nc = get_nc()
    last_err = None
    for _attempt in range(3):
        try:
            res = run_bass_kernel_spmd(nc, in_maps, core_ids=list(range(N_CORES)),
                                       **run_kwargs)
            break
        except Exception as e:  # transient NRT device errors — retry
            last_err = e
    else:
        raise last_err
    _CACHE["last_results"] = res
    out = np.concatenate(
        [r["out"].reshape(B_PER_CORE, VALID_T, K) for r in res.results], axis=0
    )
    return out.astype(np.float32, copy=False)



# revision 2
# speedup vs baseline: 1.0004x; 1.0004x over previous
"""Batch-assign-probability (VQ codebook softmax) kernel for 8 Trainium2 cores.

Math: for each valid row x (D=512), over K=256 centers c_k:
    softmax_k(-||x - c_k||^2) == softmax_k(2 x.c_k - ||c_k||^2)
(the ||x||^2 term is constant over k and cancels in softmax).

Sharding: batch B=16 split across 8 cores (2 batches = 2048 valid rows per
core); the small centers table is replicated.

Precision scheme (fp16 matmuls, full PE rate):
  1-pass: logits ~= xh.ch           (xh=fp16(x), ch=fp16(2c^T))
  2-pass: pass A = xh.ch, pass B = a2.b2 with
              a2 = fp16(xl + xh/S),  b2 = fp16(ch + S*cl)
          so A+B = (1+1/S) xh.ch + xl.ch + xh.cl + O(S*xl.cl).
          The (1+1/S) surplus is removed exactly by the ACT exp()'s scale
          parameter (scale = S/(S+1)); the -||c||^2 bias is pre-divided by
          scale on host. Emulated max-abs softmax error ~4.5e-4 (S=128).
Output is written fp16 and upcast to f32 on host (adds ~2e-4 abs err).

Device (per core, Tile framework):
  - load ct planes + aux(bias|ones) once; stream x in row-groups; per
    128-row subtile: 4*H fp16 matmuls + 1 bias matmul -> PSUM logits
    [128,256]; per 2-subtile pair: reduce_max(negate) (-> scale mul) ->
    ACT exp(scale, bias=-scale*max, accum sum); per group: reciprocal ->
    broadcast multiply (fp16 out) -> group out DMA on alternating rings.
"""

import numpy as np

import concourse.bacc as bacc
import concourse.tile as tile
from concourse import mybir
from concourse.bass_utils import run_bass_kernel_spmd

B, T, W, C, K = 16, 2048, 512, 1, 256
VALID_T = 1024
D = W * C                       # 512
N_CORES = 8
B_PER_CORE = B // N_CORES       # 2
ROWS = B_PER_CORE * VALID_T     # 2048 rows per core
P = 128
D_CHUNKS = D // P               # 4
GROUPS = [128, 384, 512, 512, 384, 128]   # rows per x/out DMA group
N_WARM_MM = 12                 # dummy matmuls to lift the PE HAM clock-gate

N_PASSES = 2                   # 1 = xh.ch only; 2 = + composite correction
S_COMP = 128.0                 # composite split scale
SCALE = S_COMP / (S_COMP + 1.0) if N_PASSES == 2 else 1.0

assert sum(GROUPS) == ROWS
X_TOTAL = P * N_PASSES * D_CHUNKS * ROWS   # flat fp16 element count of x param
AUXW = K + P                               # bias row block | ones block

F16_NP = np.float16

_CACHE: dict = {}


def _build_bass():
    f32 = mybir.dt.float32
    f16 = mybir.dt.float16
    H = N_PASSES
    nc = bacc.Bacc()
    # x planes (hi, composite), group-major, fully contiguous per group:
    # for each group g (R rows), block [128p, H, 4c, R] flattened.
    xp = nc.declare_dram_parameter("xp", [X_TOTAL], f16, isOutput=False)
    # ct plane blocks, each [128p, 4c, 256k] contiguous.
    ctp = nc.declare_dram_parameter("ctp", [H * P * D_CHUNKS * K], f16,
                                    isOutput=False)
    # aux = bias rows [P, K] | ones [P, P]
    aux = nc.declare_dram_parameter("aux", [P, AUXW], f16, isOutput=False)
    out = nc.declare_dram_parameter("out", [ROWS, K], f16, isOutput=True)

    out_v = out.rearrange("(t p) k -> p t k", p=P)       # [128, 16, 256]
    ct_plane = P * D_CHUNKS * K

    with tile.TileContext(nc) as tc:
        with (
            tc.tile_pool(name="singles", bufs=1) as singles,
            tc.tile_pool(name="xpool", bufs=1) as xpool,
            tc.tile_pool(name="opool", bufs=3) as opool,
            tc.tile_pool(name="small", bufs=8) as small,
            tc.tile_pool(name="psum", bufs=7, space="PSUM") as psum,
            tc.tile_pool(name="psum_warm", bufs=1, space="PSUM") as psum_warm,
        ):
            # Two HWDGE rings: scalar ring carries ct planes + aux (+ g0's
            # composite plane); sync ring carries the x groups so the first
            # x wire overlaps ct's instead of queueing behind it.
            ct_sb = singles.tile([P, H, D_CHUNKS, K], f16)
            xgs = []
            xoff = 0

            def x_dma(g, R, split=False):
                xg = xpool.tile([P, H, D_CHUNKS, R], f16, tag=f"xg{g}")
                n = P * H * D_CHUNKS * R
                src = xp[xoff:xoff + n].rearrange(
                    "(p h c r) -> p h c r", p=P, h=H, c=D_CHUNKS)
                if split and H == 2:
                    # hi plane on the sync ring (gates the first matmuls),
                    # composite plane in parallel on the scalar ring
                    nc.sync.dma_start(out=xg[:, 0], in_=src[:, 0])
                    nc.scalar.dma_start(out=xg[:, 1], in_=src[:, 1])
                else:
                    nc.sync.dma_start(out=xg[:], in_=src)
                xgs.append(xg)
                return n

            def ct_dma(h):
                nc.scalar.dma_start(
                    out=ct_sb[:, h],
                    in_=ctp[h * ct_plane:(h + 1) * ct_plane].rearrange(
                        "(p c k) -> p c k", p=P, c=D_CHUNKS),
                )

            ct_dma(0)
            xoff += x_dma(0, GROUPS[0], split=True)
            if H == 2:
                ct_dma(1)
            aux_sb = singles.tile([P, AUXW], f16)
            nc.scalar.dma_start(out=aux_sb[:], in_=aux[:])
            bias_sb = aux_sb[:, :K]
            ones_sb = aux_sb[:, K:]
            for g, R in enumerate(GROUPS[1:], start=1):
                xoff += x_dma(g, R)

            # PE warm-up: dummy matmuls on scratch data keep the PE busy
            # through the HAM activity window while the first x DMA lands,
            # so the real matmul stream runs at 2.4 GHz from the start.
            warm_sb = singles.tile([P, 256], f16)
            nc.gpsimd.memset(warm_sb[:], 0.0)
            warm_ps = psum_warm.tile([P, 256], f32, tag="warm")
            for _ in range(N_WARM_MM):
                nc.tensor.matmul(
                    warm_ps[:], lhsT=warm_sb[:, :P], rhs=warm_sb[:],
                    start=True, stop=True,
                )

            t0 = 0  # running 128-row tile index
            for g, R in enumerate(GROUPS):
                xg = xgs[g]
                subtiles = R // P
                og = opool.tile([P, subtiles, K], f16, tag="og")
                esum_g = small.tile([P, subtiles], f32, tag="esum")
                # pair subtiles into one full PSUM bank: shared reduce_max,
                # per-group reciprocal + broadcast multiply
                for s0 in range(0, subtiles, 2):
                    pair = min(2, subtiles - s0)
                    ps = psum.tile([P, pair, K], f32, tag="ps")
                    for j in range(pair):
                        s = s0 + j
                        rsl = slice(s * P, (s + 1) * P)
                        first = True
                        for h in range(H):
                            for c in range(D_CHUNKS):
                                nc.tensor.matmul(
                                    ps[:, j, :],
                                    lhsT=xg[:, h, c, rsl],
                                    rhs=ct_sb[:, h, c, :],
                                    start=first,
                                    stop=False,
                                )
                                first = False
                        nc.tensor.matmul(
                            ps[:, j, :],
                            lhsT=ones_sb,
                            rhs=bias_sb,
                            start=False,
                            stop=True,
                        )
                    negm = small.tile([P, pair], f32, tag="negm")
                    nc.vector.reduce_max(
                        out=negm[:], in_=ps[:], axis=mybir.AxisListType.X,
                        negate=True,
                    )
                    if SCALE != 1.0:
                        nc.vector.tensor_scalar_mul(negm[:], negm[:], SCALE)
                    for j in range(pair):
                        nc.scalar.activation(
                            out=og[:, s0 + j, :],
                            in_=ps[:, j, :],
                            func=mybir.ActivationFunctionType.Exp,
                            bias=negm[:, j:j + 1],
                            scale=SCALE,
                            accum_out=esum_g[:, s0 + j:s0 + j + 1],
                        )
                rinv_g = small.tile([P, subtiles], f32, tag="rinv")
                nc.vector.reciprocal(out=rinv_g[:], in_=esum_g[:])
                nc.vector.tensor_mul(
                    og[:],
                    og[:],
                    rinv_g[:, :, None].broadcast_to([P, subtiles, K]),
                )
                eng = nc.sync if g % 2 == 0 else nc.scalar
                eng.dma_start(out=out_v[:, t0:t0 + subtiles, :], in_=og[:])
                t0 += subtiles
    nc.finalize()
    return nc


def get_nc():
    if "nc" not in _CACHE:
        _CACHE["nc"] = _build_bass()
    return _CACHE["nc"]


def prep_inputs(y_pred: np.ndarray, mask: np.ndarray, centers: np.ndarray):
    """Host-side shard prep: valid-timestep slice, per-core transpose,
    fp16 (+ composite) planes, contiguous per-DMA packing."""
    x = np.ascontiguousarray(y_pred.reshape(B, T, D))
    masktime = np.asarray(mask).reshape(B, T, D)[0, :, 0]
    valid_idx = np.nonzero(masktime == 0)[0][:VALID_T]
    assert valid_idx.shape[0] == VALID_T
    if valid_idx[0] == 0 and valid_idx[-1] == VALID_T - 1:
        xv = x[:, :VALID_T]                    # [B, VALID_T, D]
    else:
        xv = x[:, valid_idx]

    centers = np.asarray(centers, dtype=np.float32)
    ct = (2.0 * centers).T.astype(np.float32)               # [D, K]
    ch = ct.astype(F16_NP)
    planes = [ch]
    if N_PASSES == 2:
        cl = ct - ch.astype(np.float32)
        planes.append((ch.astype(np.float32) + S_COMP * cl).astype(F16_NP))
    # [c, p, k] -> [p, c, k] contiguous per plane
    ct_blocks = [
        np.ascontiguousarray(h.reshape(D_CHUNKS, P, K).transpose(1, 0, 2)).ravel()
        for h in planes
    ]
    ctp = np.ascontiguousarray(np.concatenate(ct_blocks))

    negc2 = -(centers.astype(np.float64) ** 2).sum(axis=1)  # [K]
    bias_pre = negc2 / SCALE
    b1 = bias_pre.astype(F16_NP)
    r1 = bias_pre - b1.astype(np.float64)
    b2 = r1.astype(F16_NP)
    b3 = (r1 - b2.astype(np.float64)).astype(F16_NP)
    aux = np.zeros((P, AUXW), dtype=F16_NP)
    aux[0, :K], aux[1, :K], aux[2, :K] = b1, b2, b3
    aux[:, K:] = np.ones((P, P), dtype=F16_NP)

    in_maps = []
    for core in range(N_CORES):
        xc = xv[core * B_PER_CORE:(core + 1) * B_PER_CORE].reshape(ROWS, D)
        xTc = np.ascontiguousarray(xc.T).astype(np.float32)  # [D, ROWS]
        xh = xTc.astype(F16_NP)
        xplanes = [xh]
        if N_PASSES == 2:
            xl = xTc - xh.astype(np.float32)
            xplanes.append(
                (xl + xh.astype(np.float32) / S_COMP).astype(F16_NP))
        # [h, c, p, row] -> [p, h, c, row]
        base = np.stack(xplanes).reshape(
            N_PASSES, D_CHUNKS, P, ROWS).transpose(2, 0, 1, 3)
        blocks = []
        r0 = 0
        for R in GROUPS:
            blocks.append(np.ascontiguousarray(base[:, :, :, r0:r0 + R]).ravel())
            r0 += R
        xp = np.concatenate(blocks)
        assert xp.shape[0] == X_TOTAL
        in_maps.append({"xp": xp, "ctp": ctp, "aux": aux})
    return in_maps


def kernel(y_pred: np.ndarray, mask: np.ndarray, centers: np.ndarray,
           **run_kwargs) -> np.ndarray:
    in_maps = prep_inputs(y_pred, mask, centers)
    nc = get_nc()
    last_err = None
    for _attempt in range(3):
        try:
            res = run_bass_kernel_spmd(nc, in_maps, core_ids=list(range(N_CORES)),
                                       **run_kwargs)
            break
        except Exception as e:  # transient NRT device errors — retry
            last_err = e
    else:
        raise last_err
    _CACHE["last_results"] = res
    out = np.concatenate(
        [r["out"].astype(np.float32).reshape(B_PER_CORE, VALID_T, K)
         for r in res.results], axis=0
    )
    return out


# revision 10
# speedup vs baseline: 1.0064x; 1.0059x over previous
"""Batch-assign-probability (VQ codebook softmax) kernel for 8 Trainium2 cores.

Math: for each valid row x (D=512), over K=256 centers c_k:
    softmax_k(-||x - c_k||^2) == softmax_k(2 x.c_k - ||c_k||^2)
(the ||x||^2 term is constant over k and cancels in softmax).

Sharding: batch B=16 split across 8 cores (2 batches = 2048 valid rows per
core); the small centers table is replicated.

Precision scheme (fp16 matmuls, full PE rate):
  1-pass: logits ~= xh.ch           (xh=fp16(x), ch=fp16(2c^T))
  2-pass: pass A = xh.ch, pass B = a2.b2 with
              a2 = fp16(xl + xh/S),  b2 = fp16(ch + S*cl)
          so A+B = (1+1/S) xh.ch + xl.ch + xh.cl + O(S*xl.cl).
          The (1+1/S) surplus is removed exactly by the ACT exp()'s scale
          parameter (scale = S/(S+1)); the -||c||^2 bias is pre-divided by
          scale on host. Emulated max-abs softmax error ~4.5e-4 (S=128).
Output is written fp16 and upcast to f32 on host.

DMA plan: two independent delivery paths run in parallel (HWDGE sync ring
and SWDGE gpsimd ring share the 16 SDMA engines via per-packet round-robin
of separate internal queues): sync carries ct planes + 3-row bias + odd x
groups, gpsimd carries even x groups. ones for the bias matmul is memset on
device; out DMAs alternate rings so the scalar (ACT) engine only runs exps.
x groups are 256 rows (512 KB) so completion semaphores track byte arrival
closely and the PE pipelines group-by-group without multi-us sem stalls.
"""

import numpy as np

import concourse.bacc as bacc
import concourse.tile as tile
from concourse import mybir
from concourse.bass_utils import run_bass_kernel_spmd

B, T, W, C, K = 16, 2048, 512, 1, 256
VALID_T = 1024
D = W * C                       # 512
N_CORES = 8
B_PER_CORE = B // N_CORES       # 2
ROWS = B_PER_CORE * VALID_T     # 2048 rows per core
P = 128
D_CHUNKS = D // P               # 4
GROUPS = [128] + [256] * 7 + [128]    # rows per x/out DMA group
N_WARM_MM = 4                  # N=512 dummy matmuls against the HAM gate

N_PASSES = 2                   # 1 = xh.ch only; 2 = + composite correction
S_COMP = 128.0                 # composite split scale
SCALE = S_COMP / (S_COMP + 1.0) if N_PASSES == 2 else 1.0

assert sum(GROUPS) == ROWS
T_TILES = ROWS // P                        # 16
X_TOTAL = P * N_PASSES * D_CHUNKS * ROWS   # flat fp16 element count of x param
BIAS_ROWS = 3

F16_NP = np.float16

_CACHE: dict = {}


def _build_bass():
    f32 = mybir.dt.float32
    f16 = mybir.dt.float16
    H = N_PASSES
    nc = bacc.Bacc()
    # x planes (hi, composite), group-major, fully contiguous per group:
    # for each group g (R rows), block [128p, H, 4c, R] flattened.
    xp = nc.declare_dram_parameter("xp", [X_TOTAL], f16, isOutput=False)
    # ct plane blocks, each [128p, 4c, 256k] contiguous.
    ctp = nc.declare_dram_parameter("ctp", [H * P * D_CHUNKS * K], f16,
                                    isOutput=False)
    biasp = nc.declare_dram_parameter("biasp", [BIAS_ROWS, K], f16,
                                      isOutput=False)
    # out[p, t*K + k] = exp(logit - max) for row = t*128 + p (unnormalized);
    # esum[p, t] = sum_k exp. Host divides and transposes back.
    out = nc.declare_dram_parameter("out", [P, T_TILES * K], f16,
                                    isOutput=True)
    esum_out = nc.declare_dram_parameter("esum", [P, T_TILES], f32,
                                         isOutput=True)

    out_v = out.rearrange("p (t k) -> p t k", k=K)       # [128, 16, 256]
    ct_plane = P * D_CHUNKS * K

    with tile.TileContext(nc) as tc:
        with (
            tc.tile_pool(name="singles", bufs=1) as singles,
            tc.tile_pool(name="xpool", bufs=1) as xpool,
            tc.tile_pool(name="opool", bufs=3) as opool,
            tc.tile_pool(name="small", bufs=8) as small,
            tc.tile_pool(name="psum", bufs=7, space="PSUM") as psum,
            tc.tile_pool(name="psum_warm", bufs=1, space="PSUM") as psum_warm,
        ):
            ct_sb = singles.tile([P, H, D_CHUNKS, K], f16)
            bias_sb = singles.tile([P, K], f16)
            ones_sb = singles.tile([P, P], f16)
            warm_sb = singles.tile([P, 512], f16)
            esum_all = singles.tile([P, T_TILES], f32)
            # device-made constants (no DMA): ones for the bias matmul,
            # zeros below the 3 real bias rows, warmup scratch
            nc.gpsimd.memset(warm_sb[:], 0.0)
            nc.gpsimd.memset(ones_sb[:], 1.0)
            nc.gpsimd.memset(bias_sb[:], 0.0)

            xgs = []
            xoff = 0

            def x_dma(g, R, eng, split=False):
                xg = xpool.tile([P, H, D_CHUNKS, R], f16, tag=f"xg{g}")
                n = P * H * D_CHUNKS * R
                src = xp[xoff:xoff + n].rearrange(
                    "(p h c r) -> p h c r", p=P, h=H, c=D_CHUNKS)
                if split and H == 2:
                    eng.dma_start(out=xg[:, 0], in_=src[:, 0])
                    eng.dma_start(out=xg[:, 1], in_=src[:, 1])
                else:
                    eng.dma_start(out=xg[:], in_=src)
                xgs.append(xg)
                return n

            def ct_dma(h):
                nc.sync.dma_start(
                    out=ct_sb[:, h],
                    in_=ctp[h * ct_plane:(h + 1) * ct_plane].rearrange(
                        "(p c k) -> p c k", p=P, c=D_CHUNKS),
                )

            # sync ring FIFO: ct0, bias(1.5KB), xg0 (split planes), ct1,
            # then odd groups. gpsimd (SWDGE): even groups, in parallel.
            ct_dma(0)
            nc.sync.dma_start(out=bias_sb[:BIAS_ROWS, :], in_=biasp[:])
            xoff += x_dma(0, GROUPS[0], nc.sync, split=True)
            if H == 2:
                ct_dma(1)
            for g, R in enumerate(GROUPS[1:], start=1):
                eng = nc.sync if g % 2 == 1 else nc.gpsimd
                xoff += x_dma(g, R, eng)

            # PE warm-up: dummy matmuls on scratch data keep the PE busy
            # through the HAM activity window while the first x DMA lands.
            warm_ps = psum_warm.tile([P, 512], f32, tag="warm")
            for _ in range(N_WARM_MM):
                nc.tensor.matmul(
                    warm_ps[:], lhsT=warm_sb[:, :P], rhs=warm_sb[:],
                    start=True, stop=True,
                )

            t0 = 0  # running 128-row tile index
            for g, R in enumerate(GROUPS):
                xg = xgs[g]
                subtiles = R // P
                og = opool.tile([P, subtiles, K], f16, tag="og")
                for s0 in range(0, subtiles, 2):
                    pair = min(2, subtiles - s0)
                    ps = psum.tile([P, pair, K], f32, tag="ps")
                    # NOTE: keep each subtile's accumulation group contiguous
                    # — a start=True clears has_written for the whole bank,
                    # so interleaving two in-flight groups in one bank breaks
                    # the first one's accumulation.
                    for j in range(pair):
                        rsl = slice((s0 + j) * P, (s0 + j + 1) * P)
                        first = True
                        for h in range(H):
                            for c in range(D_CHUNKS):
                                nc.tensor.matmul(
                                    ps[:, j, :],
                                    lhsT=xg[:, h, c, rsl],
                                    rhs=ct_sb[:, h, c, :],
                                    start=first,
                                    stop=False,
                                )
                                first = False
                        nc.tensor.matmul(
                            ps[:, j, :],
                            lhsT=ones_sb[:],
                            rhs=bias_sb[:],
                            start=False,
                            stop=True,
                        )
                    negm = small.tile([P, pair], f32, tag="negm")
                    nc.vector.reduce_max(
                        out=negm[:], in_=ps[:], axis=mybir.AxisListType.X,
                        negate=True,
                    )
                    if SCALE != 1.0:
                        nc.vector.tensor_scalar_mul(negm[:], negm[:], SCALE)
                    for j in range(pair):
                        t = t0 + s0 + j
                        nc.scalar.activation(
                            out=og[:, s0 + j, :],
                            in_=ps[:, j, :],
                            func=mybir.ActivationFunctionType.Exp,
                            bias=negm[:, j:j + 1],
                            scale=SCALE,
                            accum_out=esum_all[:, t:t + 1],
                        )
                # out rings: alternate sync/gpsimd; keep the scalar (ACT)
                # engine free for exps. Last group rides the fast HWDGE.
                eng = nc.sync if g % 2 == 0 else nc.gpsimd
                if g == len(GROUPS) - 1:
                    eng = nc.sync
                eng.dma_start(out=out_v[:, t0:t0 + subtiles, :], in_=og[:])
                t0 += subtiles
            nc.gpsimd.dma_start(out=esum_out[:], in_=esum_all[:])
    nc.finalize()
    return nc


def get_nc():
    if "nc" not in _CACHE:
        _CACHE["nc"] = _build_bass()
    return _CACHE["nc"]


def prep_inputs(y_pred: np.ndarray, mask: np.ndarray, centers: np.ndarray):
    """Host-side shard prep: valid-timestep slice, per-core transpose,
    fp16 (+ composite) planes, contiguous per-DMA packing."""
    x = np.ascontiguousarray(y_pred.reshape(B, T, D))
    masktime = np.asarray(mask).reshape(B, T, D)[0, :, 0]
    valid_idx = np.nonzero(masktime == 0)[0][:VALID_T]
    assert valid_idx.shape[0] == VALID_T
    if valid_idx[0] == 0 and valid_idx[-1] == VALID_T - 1:
        xv = x[:, :VALID_T]                    # [B, VALID_T, D]
    else:
        xv = x[:, valid_idx]

    centers = np.asarray(centers, dtype=np.float32)
    ct = (2.0 * centers).T.astype(np.float32)               # [D, K]
    ch = ct.astype(F16_NP)
    planes = [ch]
    if N_PASSES == 2:
        cl = ct - ch.astype(np.float32)
        planes.append((ch.astype(np.float32) + S_COMP * cl).astype(F16_NP))
    # [c, p, k] -> [p, c, k] contiguous per plane
    ct_blocks = [
        np.ascontiguousarray(h.reshape(D_CHUNKS, P, K).transpose(1, 0, 2)).ravel()
        for h in planes
    ]
    ctp = np.ascontiguousarray(np.concatenate(ct_blocks))

    negc2 = -(centers.astype(np.float64) ** 2).sum(axis=1)  # [K]
    bias_pre = negc2 / SCALE
    biasp = np.zeros((BIAS_ROWS, K), dtype=F16_NP)
    rem = bias_pre
    for i in range(BIAS_ROWS):
        biasp[i] = rem.astype(F16_NP)
        rem = rem - biasp[i].astype(np.float64)

    in_maps = []
    for core in range(N_CORES):
        xc = xv[core * B_PER_CORE:(core + 1) * B_PER_CORE].reshape(ROWS, D)
        xTc = np.ascontiguousarray(xc.T).astype(np.float32)  # [D, ROWS]
        xh = xTc.astype(F16_NP)
        xplanes = [xh]
        if N_PASSES == 2:
            xl = xTc - xh.astype(np.float32)
            xplanes.append(
                (xl + xh.astype(np.float32) / S_COMP).astype(F16_NP))
        # [h, c, p, row] -> [p, h, c, row]
        base = np.stack(xplanes).reshape(
            N_PASSES, D_CHUNKS, P, ROWS).transpose(2, 0, 1, 3)
        blocks = []
        r0 = 0
        for R in GROUPS:
            blocks.append(np.ascontiguousarray(base[:, :, :, r0:r0 + R]).ravel())
            r0 += R
        xp = np.concatenate(blocks)
        assert xp.shape[0] == X_TOTAL
        in_maps.append({"xp": xp, "ctp": ctp, "biasp": biasp})
    return in_maps


def kernel(y_pred: np.ndarray, mask: np.ndarray, centers: np.ndarray,
           **run_kwargs) -> np.ndarray:
    in_maps = prep_inputs(y_pred, mask, centers)
    nc = get_nc()
    last_err = None
    for _attempt in range(3):
        try:
            res = run_bass_kernel_spmd(nc, in_maps, core_ids=list(range(N_CORES)),
                                       **run_kwargs)
            break
        except Exception as e:  # transient NRT device errors — retry
            last_err = e
    else:
        raise last_err
    _CACHE["last_results"] = res
    outs = []
    for r in res.results:
        e = r["out"].astype(np.float32).reshape(P, T_TILES, K)
        e /= r["esum"].reshape(P, T_TILES, 1)
        outs.append(e.transpose(1, 0, 2).reshape(B_PER_CORE, VALID_T, K))
    return np.concatenate(outs, axis=0)


# revision 13
# speedup vs baseline: 1.4394x; 1.4303x over previous
"""Batch-assign-probability (VQ codebook softmax) kernel for 8 Trainium2 cores.

Math: for each valid row x (D=512), over K=256 centers c_k:
    softmax_k(-||x - c_k||^2) == softmax_k(2 x.c_k - ||c_k||^2)
(the ||x||^2 term is constant over k and cancels in softmax).

Sharding: batch B=16 split across 8 cores (2 batches = 2048 valid rows per
core); the small centers table is replicated.

Precision scheme (fp16 matmuls, full PE rate):
  1-pass: logits ~= xh.ch           (xh=fp16(x), ch=fp16(2c^T))
  2-pass: pass A = xh.ch, pass B = a2.b2 with
              a2 = fp16(xl + xh/S),  b2 = fp16(ch + S*cl)
          so A+B = (1+1/S) xh.ch + xl.ch + xh.cl + O(S*xl.cl).
          The (1+1/S) surplus is removed exactly by the ACT exp()'s scale
          parameter (scale = S/(S+1)); the -||c||^2 bias is pre-divided by
          scale on host. Emulated max-abs softmax error ~4.5e-4 (S=128).
Output is written fp16 and upcast to f32 on host.

DMA plan: two independent delivery paths run in parallel (HWDGE sync ring
and SWDGE gpsimd ring share the 16 SDMA engines via per-packet round-robin
of separate internal queues): sync carries ct planes + 3-row bias + odd x
groups, gpsimd carries even x groups. ones for the bias matmul is memset on
device; out DMAs alternate rings so the scalar (ACT) engine only runs exps.
x groups are 256 rows (512 KB) so completion semaphores track byte arrival
closely and the PE pipelines group-by-group without multi-us sem stalls.
"""

import numpy as np

import concourse.bacc as bacc
import concourse.tile as tile
from concourse import mybir
from concourse.bass_utils import run_bass_kernel_spmd

B, T, W, C, K = 16, 2048, 512, 1, 256
VALID_T = 1024
D = W * C                       # 512
N_CORES = 8
B_PER_CORE = B // N_CORES       # 2
ROWS = B_PER_CORE * VALID_T     # 2048 rows per core
P = 128
D_CHUNKS = D // P               # 4
GROUPS = [128] + [384] * 5            # rows per x/out DMA group
N_WARM_MM = 4                  # N=512 dummy matmuls against the HAM gate

N_PASSES = 1                   # 1 = xh.ch only; 2 = + composite correction
S_COMP = 128.0                 # composite split scale
SCALE = S_COMP / (S_COMP + 1.0) if N_PASSES == 2 else 1.0

assert sum(GROUPS) == ROWS
T_TILES = ROWS // P                        # 16
X_TOTAL = P * N_PASSES * D_CHUNKS * ROWS   # flat fp16 element count of x param
BIAS_ROWS = 3

F16_NP = np.float16

_CACHE: dict = {}


def _build_bass():
    f32 = mybir.dt.float32
    f16 = mybir.dt.float16
    H = N_PASSES
    nc = bacc.Bacc()
    # x planes (hi, composite), group-major, fully contiguous per group:
    # for each group g (R rows), block [128p, H, 4c, R] flattened.
    xp = nc.declare_dram_parameter("xp", [X_TOTAL], f16, isOutput=False)
    # ct plane blocks, each [128p, 4c, 256k] contiguous.
    ctp = nc.declare_dram_parameter("ctp", [H * P * D_CHUNKS * K], f16,
                                    isOutput=False)
    biasp = nc.declare_dram_parameter("biasp", [BIAS_ROWS, K], f16,
                                      isOutput=False)
    # out[p, t*K + k] = exp(logit - max) for row = t*128 + p (unnormalized);
    # esum[p, t] = sum_k exp. Host divides and transposes back.
    out = nc.declare_dram_parameter("out", [P, T_TILES * K], f16,
                                    isOutput=True)
    esum_out = nc.declare_dram_parameter("esum", [P, T_TILES], f32,
                                         isOutput=True)

    out_v = out.rearrange("p (t k) -> p t k", k=K)       # [128, 16, 256]
    ct_plane = P * D_CHUNKS * K

    with tile.TileContext(nc) as tc:
        with (
            tc.tile_pool(name="singles", bufs=1) as singles,
            tc.tile_pool(name="xpool", bufs=1) as xpool,
            tc.tile_pool(name="opool", bufs=3) as opool,
            tc.tile_pool(name="small", bufs=8) as small,
            tc.tile_pool(name="psum", bufs=7, space="PSUM") as psum,
            tc.tile_pool(name="psum_warm", bufs=1, space="PSUM") as psum_warm,
        ):
            ct_sb = singles.tile([P, H, D_CHUNKS, K], f16)
            bias_sb = singles.tile([P, K], f16)
            ones_sb = singles.tile([P, P], f16)
            warm_sb = singles.tile([P, 512], f16)
            esum_all = singles.tile([P, T_TILES], f32)
            # device-made constants (no DMA): ones for the bias matmul,
            # zeros below the 3 real bias rows, warmup scratch
            nc.gpsimd.memset(warm_sb[:], 0.0)
            nc.gpsimd.memset(ones_sb[:], 1.0)
            nc.gpsimd.memset(bias_sb[:], 0.0)

            xgs = []
            xoff = 0

            def x_dma(g, R):
                xg = xpool.tile([P, H, D_CHUNKS, R], f16, tag=f"xg{g}")
                n = P * H * D_CHUNKS * R
                src = xp[xoff:xoff + n].rearrange(
                    "(p h c r) -> p h c r", p=P, h=H, c=D_CHUNKS)
                nc.sync.dma_start(out=xg[:], in_=src)
                xgs.append(xg)
                return n

            def ct_dma(h):
                nc.sync.dma_start(
                    out=ct_sb[:, h],
                    in_=ctp[h * ct_plane:(h + 1) * ct_plane].rearrange(
                        "(p c k) -> p c k", p=P, c=D_CHUNKS),
                )

            # Everything bulk rides the sync HWDGE ring (SWDGE concurrent
            # with HWDGE halves aggregate DMA rate; the scalar ring starves
            # behind a busy sync ring). FIFO: ct0, bias(1.5KB), xg0, [ct1],
            # then the remaining groups — per-group sems pace the PE.
            ct_dma(0)
            nc.sync.dma_start(out=bias_sb[:BIAS_ROWS, :], in_=biasp[:])
            xoff += x_dma(0, GROUPS[0])
            if H == 2:
                ct_dma(1)
            for g, R in enumerate(GROUPS[1:], start=1):
                xoff += x_dma(g, R)

            # PE warm-up: dummy matmuls on scratch data keep the PE busy
            # through the HAM activity window while the first x DMA lands.
            warm_ps = psum_warm.tile([P, 512], f32, tag="warm")
            for _ in range(N_WARM_MM):
                nc.tensor.matmul(
                    warm_ps[:], lhsT=warm_sb[:, :P], rhs=warm_sb[:],
                    start=True, stop=True,
                )

            t0 = 0  # running 128-row tile index
            for g, R in enumerate(GROUPS):
                xg = xgs[g]
                subtiles = R // P
                og = opool.tile([P, subtiles, K], f16, tag="og")
                for s0 in range(0, subtiles, 2):
                    pair = min(2, subtiles - s0)
                    ps = psum.tile([P, pair, K], f32, tag="ps")
                    # NOTE: keep each subtile's accumulation group contiguous
                    # — a start=True clears has_written for the whole bank,
                    # so interleaving two in-flight groups in one bank breaks
                    # the first one's accumulation.
                    for j in range(pair):
                        rsl = slice((s0 + j) * P, (s0 + j + 1) * P)
                        first = True
                        for h in range(H):
                            for c in range(D_CHUNKS):
                                nc.tensor.matmul(
                                    ps[:, j, :],
                                    lhsT=xg[:, h, c, rsl],
                                    rhs=ct_sb[:, h, c, :],
                                    start=first,
                                    stop=False,
                                )
                                first = False
                        nc.tensor.matmul(
                            ps[:, j, :],
                            lhsT=ones_sb[:],
                            rhs=bias_sb[:],
                            start=False,
                            stop=True,
                        )
                    negm = small.tile([P, pair], f32, tag="negm")
                    nc.vector.reduce_max(
                        out=negm[:], in_=ps[:], axis=mybir.AxisListType.X,
                        negate=True,
                    )
                    if SCALE != 1.0:
                        nc.vector.tensor_scalar_mul(negm[:], negm[:], SCALE)
                    for j in range(pair):
                        t = t0 + s0 + j
                        nc.scalar.activation(
                            out=og[:, s0 + j, :],
                            in_=ps[:, j, :],
                            func=mybir.ActivationFunctionType.Exp,
                            bias=negm[:, j:j + 1],
                            scale=SCALE,
                            accum_out=esum_all[:, t:t + 1],
                        )
                # early groups' outs trickle on the scalar ring while sync
                # still streams x; late outs ride sync right behind the x
                # FIFO and drain at full rate once x is done.
                eng = nc.scalar if g < len(GROUPS) // 2 else nc.sync
                eng.dma_start(out=out_v[:, t0:t0 + subtiles, :], in_=og[:])
                t0 += subtiles
            nc.sync.dma_start(out=esum_out[:], in_=esum_all[:])
    nc.finalize()
    return nc


def get_nc():
    if "nc" not in _CACHE:
        _CACHE["nc"] = _build_bass()
    return _CACHE["nc"]


def prep_inputs(y_pred: np.ndarray, mask: np.ndarray, centers: np.ndarray):
    """Host-side shard prep: valid-timestep slice, per-core transpose,
    fp16 (+ composite) planes, contiguous per-DMA packing."""
    x = np.ascontiguousarray(y_pred.reshape(B, T, D))
    masktime = np.asarray(mask).reshape(B, T, D)[0, :, 0]
    valid_idx = np.nonzero(masktime == 0)[0][:VALID_T]
    assert valid_idx.shape[0] == VALID_T
    if valid_idx[0] == 0 and valid_idx[-1] == VALID_T - 1:
        xv = x[:, :VALID_T]                    # [B, VALID_T, D]
    else:
        xv = x[:, valid_idx]

    centers = np.asarray(centers, dtype=np.float32)
    ct = (2.0 * centers).T.astype(np.float32)               # [D, K]
    ch = ct.astype(F16_NP)
    planes = [ch]
    if N_PASSES == 2:
        cl = ct - ch.astype(np.float32)
        planes.append((ch.astype(np.float32) + S_COMP * cl).astype(F16_NP))
    # [c, p, k] -> [p, c, k] contiguous per plane
    ct_blocks = [
        np.ascontiguousarray(h.reshape(D_CHUNKS, P, K).transpose(1, 0, 2)).ravel()
        for h in planes
    ]
    ctp = np.ascontiguousarray(np.concatenate(ct_blocks))

    negc2 = -(centers.astype(np.float64) ** 2).sum(axis=1)  # [K]
    bias_pre = negc2 / SCALE
    biasp = np.zeros((BIAS_ROWS, K), dtype=F16_NP)
    rem = bias_pre
    for i in range(BIAS_ROWS):
        biasp[i] = rem.astype(F16_NP)
        rem = rem - biasp[i].astype(np.float64)

    in_maps = []
    for core in range(N_CORES):
        xc = xv[core * B_PER_CORE:(core + 1) * B_PER_CORE].reshape(ROWS, D)
        xTc = np.ascontiguousarray(xc.T).astype(np.float32)  # [D, ROWS]
        xh = xTc.astype(F16_NP)
        xplanes = [xh]
        if N_PASSES == 2:
            xl = xTc - xh.astype(np.float32)
            xplanes.append(
                (xl + xh.astype(np.float32) / S_COMP).astype(F16_NP))
        # [h, c, p, row] -> [p, h, c, row]
        base = np.stack(xplanes).reshape(
            N_PASSES, D_CHUNKS, P, ROWS).transpose(2, 0, 1, 3)
        blocks = []
        r0 = 0
        for R in GROUPS:
            blocks.append(np.ascontiguousarray(base[:, :, :, r0:r0 + R]).ravel())
            r0 += R
        xp = np.concatenate(blocks)
        assert xp.shape[0] == X_TOTAL
        in_maps.append({"xp": xp, "ctp": ctp, "biasp": biasp})
    return in_maps


def kernel(y_pred: np.ndarray, mask: np.ndarray, centers: np.ndarray,
           **run_kwargs) -> np.ndarray:
    in_maps = prep_inputs(y_pred, mask, centers)
    nc = get_nc()
    last_err = None
    for _attempt in range(3):
        try:
            res = run_bass_kernel_spmd(nc, in_maps, core_ids=list(range(N_CORES)),
                                       **run_kwargs)
            break
        except Exception as e:  # transient NRT device errors — retry
            last_err = e
    else:
        raise last_err
    _CACHE["last_results"] = res
    outs = []
    for r in res.results:
        e = r["out"].astype(np.float32).reshape(P, T_TILES, K)
        e /= r["esum"].reshape(P, T_TILES, 1)
        outs.append(e.transpose(1, 0, 2).reshape(B_PER_CORE, VALID_T, K))
    return np.concatenate(outs, axis=0)
